# revision 25
# baseline (speedup 1.0000x reference)
"""Dense dot-product attention (B=1, H=16, S=4096, D=64, fp32) on 8 trn2 cores.

Head-parallel: core c computes heads [2c, 2c+1] fully on-device, no comms.

Per-head algorithm (S^T score layout, [q,d] output layout):
  scores: S^T[k, q] PSUM tile per (k-tile, q-group) via f32r matmuls with a
    65-row contraction: rows 0:64 contract d (K pre-scaled by 128*log2e),
    row 64 is a bias row adding SROW = 128*(127 - SH) - 64, so the PSUM
    value is v = 128*(x + 127) - 64 with x = score*log2e - SH.
  exp -> P (bf16) via two engine paths, split per k-tile:
    ACT path: P = Exp(v * ln2/128 + bias), full precision, bf16 out.
    DVE+Pool path ("expbits"): DVE custom op computes
        w = (C1*_t + C2)*_t + v,  _t = v - 128*round(v/128)  (magic round)
      then Pool adds K and converts to int16; the int16 IS the bf16 bit
      pattern of 2^x (exponent from the 128-quantized part, mantissa from
      the quadratic 2^frac fit; max weight err ~1%, rel out err ~3e-3).
  PV: out[q, d'] accumulates over 32 k-tiles via bf16 matmuls with
    lhsT = P^T[k, q-subtile] (ap=65/matmul), rhs = V'[k, 0:65] where V' has
    a ones column so out[q, 64] = sum of P (softmax denominator).
  normalize: per q-subtile reciprocal + per-partition scalar multiply;
    output written in natural [q, d] layout (no transposes).

PV matmuls are emitted LAG k-tiles behind QK so the exp chain latency never
stalls the in-order PE queue.
"""

import sys

if "/opt/trn_rl_repo" not in sys.path:
    sys.path.insert(0, "/opt/trn_rl_repo")

import numpy as np

B, H, S, D = 1, 16, 4096, 64
N_CORES = 8
HPC = H // N_CORES    # heads per core = 2

KT = S // 128         # 32 k-tiles per head
NPAIR = KT // 2       # k-tiles are processed in pairs of 2
GROUP = 512           # q columns per score group
NG = S // GROUP       # 8 groups per head
NJ = GROUP // 128     # 4 q-subtiles per group

LOG2E = 1.4426950408889634
LN2 = 0.6931471805599453
SH = 46.0                                   # shift in log2 units
SROW = 128.0 * (127.0 - SH) - 64.0          # bias-row constant (J = -64)
ACT_SCALE = LN2 / 128.0
ACT_BIAS = -(127.0 - 64.0 / 128.0) * LN2    # undoes 128*( . +127) - 64

# expbits constants: v + quad(_t) + K == bf16 bits of 2^x (see numcheck)
C0_MAGIC = 1.5 * 2**30
C1_QUAD = 2.459070897941e-03
C2_QUAD = -1.995185412854e-02
K_BITS = 53.044930589134

# k-tile PAIRS handled by the expbits path (rest go to ACT). Per 2 groups:
# 19 ACT pairs, 12 DVE+Pool pairs, 1 DVE+DVE pair -> every engine <=96% of
# the PE-bound group period, so queues don't build up.
DVE_PAIRS_A = frozenset({1, 4, 6, 9, 11, 14})      # 6, Pool-finished
DVE_PAIRS_B = frozenset({0, 2, 5, 7, 9, 12, 14})   # 7, one DVE-finished
DVE_FINISH_B = 9
DVE_PAIRS_LASTG = frozenset({0, 2, 4, 6, 8, 10})   # final group: early only
LAG_ACT = 3   # in pair units
LAG_DVE = 6
DEBUG_DUMP = False

_compiled = None


def _register_expbits_op():
    import concourse.dve_ops as dve_ops
    from concourse.dve_ops import DveOp, OPS, has_src1
    from concourse.dve_spec import Spec, Src0, C0, C1, C2, lower
    from concourse.dve_uop import DveOpSpec

    if "EXPBITS_ANT" in dve_ops._SUB_OPCODE_FOR_NAME:
        return {op.name: op for op in OPS}["EXPBITS_ANT"]

    from concourse.dve_spec import Zero, maxx

    f32 = np.float32

    def ref(in0, in1, s0, s1, imm2):
        x = in0.astype(np.float32)
        r = x + f32(s0)
        s = r - f32(s0)
        t = x - s
        return np.maximum((t * f32(s1) + f32(imm2)) * t + x, f32(0.0))

    _r = Src0 + C0
    _t = Src0 - (_r - C0)
    op = DveOp(
        "EXPBITS_ANT",
        # max(.., 0): deeply negative scores (x < -127) would otherwise go
        # negative in int16 and bitcast to huge negative bf16 weights.
        Spec(body=maxx((_t * C1 + C2) * _t + Src0, Zero), reference=ref),
        subdim=False,
        uops_sha={},
    )
    OPS.append(op)
    dve_ops.CUSTOM_DVE_SPECS[op.name] = op.spec
    dve_ops._SUB_OPCODE_FOR_NAME[op.name] = (
        dve_ops._CUSTOM_DVE_ROW_BASE + len(dve_ops._SUB_OPCODE_FOR_NAME))
    for ver in ("v3", "v4"):
        try:
            compiled = DveOpSpec(
                name=op.name,
                opcode=dve_ops._SUB_OPCODE_FOR_NAME[op.name],
                uops=lower(op.spec, ver=ver),
                rd1_en=has_src1(op.spec),
            )
            op.uops_sha[ver] = compiled.sha(ver)
        except Exception:
            pass
    return op


def _build():
    import concourse.bacc as bacc
    import concourse.mybir as mybir
    import concourse.tile as tile

    op_expbits = _register_expbits_op()

    f32 = mybir.dt.float32
    f32r = mybir.dt.float32r
    bf16 = mybir.dt.bfloat16
    i16 = mybir.dt.int16

    nc = bacc.Bacc("TRN2", target_bir_lowering=False, debug=False,
                   num_devices=N_CORES)

    qT = nc.dram_tensor("qT", [HPC, D + 1, S], f32r, kind="ExternalInput")
    kT = nc.dram_tensor("kT", [HPC, D + 1, S], f32r, kind="ExternalInput")
    v = nc.dram_tensor("v", [HPC, S, D + 1], bf16, kind="ExternalInput")
    out = nc.dram_tensor("out", [HPC, S, D], f32, kind="ExternalOutput")
    dbg = None
    if DEBUG_DUMP:
        dbg = nc.dram_tensor("dbg", [HPC, NG, 128, NJ, 128], f32,
                             kind="ExternalOutput")

    with tile.TileContext(nc) as tc:
        with (
            tc.tile_pool(name="qk", bufs=2) as qk_pool,
            tc.tile_pool(name="vp", bufs=2) as vp_pool,
            tc.tile_pool(name="ptb", bufs=8) as ptb_pool,
            tc.tile_pool(name="pti", bufs=8) as pti_pool,
            tc.tile_pool(name="wk", bufs=8) as wk_pool,
            tc.tile_pool(name="osb", bufs=2) as osb_pool,
            tc.tile_pool(name="rcp", bufs=16) as rcp_pool,
            tc.tile_pool(name="small", bufs=1) as small_pool,
            tc.tile_pool(name="psum_st", bufs=3, space="PSUM") as psum_st,
            tc.tile_pool(name="psum_o", bufs=2, space="PSUM") as psum_o,
        ):
            bias_t = small_pool.tile([128, 1], f32, tag="bias")
            nc.gpsimd.memset(bias_t, ACT_BIAS)
            # dummy exp so the ACT table set loads during the input DMAs
            warm_t = small_pool.tile([128, 1], f32, tag="warm")
            nc.scalar.activation(out=warm_t, in_=bias_t,
                                 func=mybir.ActivationFunctionType.Exp,
                                 bias=bias_t[:], scale=1.0)

            # ---- input loads; first head's first tiles load in small
            # chunks so the first QK starts as early as possible, and the
            # big transfers stay off the critical DMA path. ----
            kt_ts, qt_ts, vp_ts = {}, {}, {}
            for h in range(HPC):
                kt_ts[h] = qk_pool.tile([D + 1, S], f32r, tag="kt",
                                        name=f"kt_{h}")
                qt_ts[h] = qk_pool.tile([D + 1, S], f32r, tag="qt",
                                        name=f"qt_{h}")
                vp_ts[h] = vp_pool.tile([128, KT, D + 1], bf16, tag="vp",
                                        name=f"vp_{h}")
            nc.sync.dma_start(out=kt_ts[0][:, 0:256], in_=kT[0][:, 0:256])
            nc.scalar.dma_start(out=qt_ts[0][:, 0:512], in_=qT[0][:, 0:512])
            nc.sync.dma_start(out=kt_ts[0][:, 256:1024],
                              in_=kT[0][:, 256:1024])
            nc.sync.dma_start(out=kt_ts[0][:, 1024:2560],
                              in_=kT[0][:, 1024:2560])
            nc.sync.dma_start(out=kt_ts[0][:, 2560:S], in_=kT[0][:, 2560:S])
            nc.scalar.dma_start(
                out=vp_ts[0],
                in_=v[0].rearrange("(kt p) e -> p kt e", p=128))
            nc.sync.dma_start(out=qt_ts[0][:, 512:S], in_=qT[0][:, 512:S])
            nc.sync.dma_start(out=kt_ts[1], in_=kT[1])
            nc.sync.dma_start(out=qt_ts[1], in_=qT[1])
            nc.scalar.dma_start(
                out=vp_ts[1],
                in_=v[1].rearrange("(kt p) e -> p kt e", p=128))

            # ---- flat pipeline over (h, g, pair) ----
            steps = [(h, g, pp)
                     for h in range(HPC) for g in range(NG)
                     for pp in range(NPAIR)]
            pending = []          # (release_at_step, h, g, pp, pt_pair_ap)
            group_state = {}      # (h, g) -> dict(o, started, flushed)

            def get_group(h, g):
                key = (h, g)
                if key not in group_state:
                    group_state[key] = {
                        "o": psum_o.tile([128, NJ, 128], f32, tag="o",
                                         name=f"o_{h}_{g}"),
                        "bank_started": False,
                        "flushed": 0,
                    }
                return group_state[key]

            def emit_pv(h, g, pp, pt_b):
                gs = get_group(h, g)
                o_ps = gs["o"]
                for half in range(2):
                    kt = 2 * pp + half
                    last = (kt == KT - 1)
                    for j in range(NJ):
                        # start=True clears has_written for the WHOLE psum
                        # bank, so it is issued exactly once per o_ps tile;
                        # later writes to virgin elements overwrite, the
                        # rest accumulate.
                        nc.tensor.matmul(
                            o_ps[:, j, 0:D + 1],
                            lhsT=pt_b[:, half, j * 128:(j + 1) * 128],
                            rhs=vp_ts[h][:, kt, :],
                            start=(not gs["bank_started"]), stop=last,
                            skip_group_check=True,
                        )
                        gs["bank_started"] = True
                gs["flushed"] += 1
                if gs["flushed"] == NPAIR:
                    emit_norm(h, g)

            def emit_norm(h, g):
                gs = group_state[(h, g)]
                o_ps = gs["o"]
                out_sb = osb_pool.tile([128, NJ, D], f32, tag="osb",
                                       name=f"osb_{h}_{g}")
                out_view = out[h][g * GROUP:(g + 1) * GROUP, :].rearrange(
                    "(j p) d -> p j d", p=128)
                last_grp = (h == HPC - 1 and g == NG - 1)
                for j in range(NJ):
                    rcp_t = rcp_pool.tile([128, 1], f32, tag="rcp",
                                          name=f"rcp_{h}_{g}_{j}")
                    nc.vector.reciprocal(out=rcp_t, in_=o_ps[:, j, D:D + 1])
                    if last_grp and j % 2 == 1:
                        # tail: split muls across ACT/DVE, per-j DMAs on
                        # two queues so the last bytes leave ASAP
                        nc.scalar.activation(
                            out=out_sb[:, j, :], in_=o_ps[:, j, 0:D],
                            func=mybir.ActivationFunctionType.Copy,
                            scale=rcp_t[:])
                    else:
                        nc.vector.tensor_scalar_mul(
                            out_sb[:, j, :], o_ps[:, j, 0:D], rcp_t)
                    if last_grp:
                        q_eng = nc.sync if j % 2 == 0 else nc.scalar
                        q_eng.dma_start(out=out_view[:, j:j + 1, :],
                                        in_=out_sb[:, j:j + 1, :])
                if not last_grp:
                    nc.sync.dma_start(out=out_view, in_=out_sb)
                del group_state[(h, g)]

            for n, (h, g, pp) in enumerate(steps):
                q0 = g * GROUP
                st_t = psum_st.tile([128, 2, GROUP], f32, tag="st",
                                    name=f"st_{h}_{g}_{pp}")
                for half in range(2):
                    kt = 2 * pp + half
                    nc.tensor.matmul(
                        st_t[:, half, :],
                        lhsT=kt_ts[h][:, kt * 128:(kt + 1) * 128],
                        rhs=qt_ts[h][:, q0:q0 + GROUP],
                        start=True, stop=True,
                    )
                # exp over the whole pair [128, 2, GROUP]
                last_grp = (h == HPC - 1 and g == NG - 1)
                if last_grp:
                    eps = DVE_PAIRS_LASTG
                else:
                    eps = DVE_PAIRS_A if (g % 2 == 0) else DVE_PAIRS_B
                if pp in eps:
                    w_t = wk_pool.tile([128, 2, GROUP], f32, tag="wk")
                    nc.vector._custom_dve(
                        op_expbits, out=w_t, in0=st_t,
                        s0=C0_MAGIC, s1=C1_QUAD, imm2=C2_QUAD)
                    pt_i = pti_pool.tile([128, 2, GROUP], i16, tag="pti")
                    if g % 2 == 1 and pp == DVE_FINISH_B and not last_grp:
                        nc.vector.tensor_scalar_add(pt_i, w_t, K_BITS)
                    else:
                        nc.gpsimd.tensor_scalar_add(pt_i, w_t, K_BITS)
                    pt_b = pt_i.bitcast(mybir.dt.bfloat16)
                    lag = LAG_DVE
                else:
                    pt_bt = ptb_pool.tile([128, 2, GROUP],
                                          mybir.dt.bfloat16, tag="ptb")
                    nc.scalar.activation(
                        out=pt_bt, in_=st_t,
                        func=mybir.ActivationFunctionType.Exp,
                        bias=bias_t[:], scale=ACT_SCALE)
                    pt_b = pt_bt
                    lag = LAG_ACT
                if last_grp and pp >= NPAIR - 3:
                    lag = 1  # shorten the kernel tail (these pairs are ACT)
                pending.append((n + lag, h, g, pp, pt_b))
                while pending and pending[0][0] <= n:
                    _, fh, fg, fpp, fpt = pending.pop(0)
                    emit_pv(fh, fg, fpp, fpt)

            while pending:
                _, fh, fg, fpp, fpt = pending.pop(0)
                emit_pv(fh, fg, fpp, fpt)

    nc.compile()
    return nc


def _get_compiled():
    global _compiled
    if _compiled is None:
        _compiled = _build()
    return _compiled


def _to_bf16(x):
    b = np.ascontiguousarray(x, np.float32).view(np.uint32)
    r = ((b >> 16) + ((b >> 15) & 1)).astype(np.uint16)
    return r


def kernel(query: np.ndarray, key: np.ndarray, value: np.ndarray) -> np.ndarray:
    import ml_dtypes
    from concourse.bass_utils import run_bass_kernel_spmd

    nc = _get_compiled()

    q = np.asarray(query, dtype=np.float32).reshape(H, S, D)
    k = np.asarray(key, dtype=np.float32).reshape(H, S, D)
    v = np.asarray(value, dtype=np.float32).reshape(H, S, D)

    ksc = np.float32(128.0 * LOG2E)
    in_maps = []
    for c in range(N_CORES):
        hs = slice(c * HPC, (c + 1) * HPC)
        qh = q[hs].transpose(0, 2, 1)                       # [HPC, 64, S]
        kh = k[hs].transpose(0, 2, 1) * ksc                 # [HPC, 64, S]
        qT_host = np.concatenate(
            [qh, np.full((HPC, 1, S), SROW, np.float32)], axis=1)
        kT_host = np.concatenate(
            [kh, np.ones((HPC, 1, S), np.float32)], axis=1)
        v_host = np.concatenate(
            [v[hs], np.ones((HPC, S, 1), np.float32)], axis=-1)
        in_maps.append({
            "qT": np.ascontiguousarray(qT_host),
            "kT": np.ascontiguousarray(kT_host),
            "v": _to_bf16(v_host).view(ml_dtypes.bfloat16),
        })

    res = run_bass_kernel_spmd(nc, in_maps, list(range(N_CORES)))

    outp = np.empty((B, H, S, D), dtype=np.float32)
    for c in range(N_CORES):
        for hh in range(HPC):
            outp[0, c * HPC + hh] = res.results[c]["out"][hh]
    return outp


# revision 27
# speedup vs baseline: 1.0076x; 1.0076x over previous
"""Dense dot-product attention (B=1, H=16, S=4096, D=64, fp32) on 8 trn2 cores.

Head-parallel: core c computes heads [2c, 2c+1] fully on-device, no comms.

Per-head algorithm (S^T score layout, [q,d] output layout):
  scores: S^T[k, q] PSUM tile per (k-tile, q-group) via f32r matmuls with a
    65-row contraction: rows 0:64 contract d (K pre-scaled by 128*log2e),
    row 64 is a bias row adding SROW = 128*(127 - SH) - 64, so the PSUM
    value is v = 128*(x + 127) - 64 with x = score*log2e - SH.
  exp -> P (bf16) via two engine paths, split per k-tile:
    ACT path: P = Exp(v * ln2/128 + bias), full precision, bf16 out.
    DVE+Pool path ("expbits"): DVE custom op computes
        w = (C1*_t + C2)*_t + v,  _t = v - 128*round(v/128)  (magic round)
      then Pool adds K and converts to int16; the int16 IS the bf16 bit
      pattern of 2^x (exponent from the 128-quantized part, mantissa from
      the quadratic 2^frac fit; max weight err ~1%, rel out err ~3e-3).
  PV: out[q, d'] accumulates over 32 k-tiles via bf16 matmuls with
    lhsT = P^T[k, q-subtile] (ap=65/matmul), rhs = V'[k, 0:65] where V' has
    a ones column so out[q, 64] = sum of P (softmax denominator).
  normalize: per q-subtile reciprocal + per-partition scalar multiply;
    output written in natural [q, d] layout (no transposes).

PV matmuls are emitted LAG k-tiles behind QK so the exp chain latency never
stalls the in-order PE queue.
"""

import sys

if "/opt/trn_rl_repo" not in sys.path:
    sys.path.insert(0, "/opt/trn_rl_repo")

import numpy as np

B, H, S, D = 1, 16, 4096, 64
N_CORES = 8
HPC = H // N_CORES    # heads per core = 2

KT = S // 128         # 32 k-tiles per head
NPAIR = KT // 2       # k-tiles are processed in pairs of 2
GROUP = 512           # q columns per score group
NG = S // GROUP       # 8 groups per head
NJ = GROUP // 128     # 4 q-subtiles per group

LOG2E = 1.4426950408889634
LN2 = 0.6931471805599453
SH = 46.0                                   # shift in log2 units
SROW = 128.0 * (127.0 - SH) - 64.0          # bias-row constant (J = -64)
ACT_SCALE = LN2 / 128.0
ACT_BIAS = -(127.0 - 64.0 / 128.0) * LN2    # undoes 128*( . +127) - 64

# expbits constants: v + quad(_t) + K == bf16 bits of 2^x (see numcheck)
C0_MAGIC = 1.5 * 2**30
C1_QUAD = 2.459070897941e-03
C2_QUAD = -1.995185412854e-02
K_BITS = 53.044930589134

# k-tile PAIRS handled by the expbits path (rest go to ACT). Pool's
# 1517ns/pair op can only sustain one every 3 pairs, so the Pool-finished
# pairs are spaced 3 apart (11 per 2 groups); 2 more pairs per 2 groups are
# DVE-finished. Every engine ends up <=95% of the PE-bound group period.
POOL_EPS_A = frozenset({0, 3, 6, 9, 12, 15})   # even groups, Pool-finished
POOL_EPS_B = frozenset({2, 5, 8, 11, 14})      # odd groups, Pool-finished
DVE_EPS_A = frozenset({13})                    # DVE-finished
DVE_EPS_B = frozenset({13})
POOL_EPS_LASTG = frozenset({0, 3, 6, 9})       # final group: early only
DVE_EPS_LASTG = frozenset({11})
LAG_ACT = 3   # in pair units
LAG_DVE = 6
DEBUG_DUMP = False

_compiled = None


def _register_expbits_op():
    import concourse.dve_ops as dve_ops
    from concourse.dve_ops import DveOp, OPS, has_src1
    from concourse.dve_spec import Spec, Src0, C0, C1, C2, lower
    from concourse.dve_uop import DveOpSpec

    if "EXPBITS_ANT" in dve_ops._SUB_OPCODE_FOR_NAME:
        return {op.name: op for op in OPS}["EXPBITS_ANT"]

    from concourse.dve_spec import Zero, maxx

    f32 = np.float32

    def ref(in0, in1, s0, s1, imm2):
        x = in0.astype(np.float32)
        r = x + f32(s0)
        s = r - f32(s0)
        t = x - s
        return np.maximum((t * f32(s1) + f32(imm2)) * t + x, f32(0.0))

    _r = Src0 + C0
    _t = Src0 - (_r - C0)
    op = DveOp(
        "EXPBITS_ANT",
        # max(.., 0): deeply negative scores (x < -127) would otherwise go
        # negative in int16 and bitcast to huge negative bf16 weights.
        Spec(body=maxx((_t * C1 + C2) * _t + Src0, Zero), reference=ref),
        subdim=False,
        uops_sha={},
    )
    OPS.append(op)
    dve_ops.CUSTOM_DVE_SPECS[op.name] = op.spec
    dve_ops._SUB_OPCODE_FOR_NAME[op.name] = (
        dve_ops._CUSTOM_DVE_ROW_BASE + len(dve_ops._SUB_OPCODE_FOR_NAME))
    for ver in ("v3", "v4"):
        try:
            compiled = DveOpSpec(
                name=op.name,
                opcode=dve_ops._SUB_OPCODE_FOR_NAME[op.name],
                uops=lower(op.spec, ver=ver),
                rd1_en=has_src1(op.spec),
            )
            op.uops_sha[ver] = compiled.sha(ver)
        except Exception:
            pass
    return op


def _build():
    import concourse.bacc as bacc
    import concourse.mybir as mybir
    import concourse.tile as tile

    op_expbits = _register_expbits_op()

    f32 = mybir.dt.float32
    f32r = mybir.dt.float32r
    bf16 = mybir.dt.bfloat16
    i16 = mybir.dt.int16

    nc = bacc.Bacc("TRN2", target_bir_lowering=False, debug=False,
                   num_devices=N_CORES)

    qT = nc.dram_tensor("qT", [HPC, D + 1, S], f32r, kind="ExternalInput")
    kT = nc.dram_tensor("kT", [HPC, D + 1, S], f32r, kind="ExternalInput")
    v = nc.dram_tensor("v", [HPC, S, D + 1], bf16, kind="ExternalInput")
    out = nc.dram_tensor("out", [HPC, S, D], f32, kind="ExternalOutput")
    dbg = None
    if DEBUG_DUMP:
        dbg = nc.dram_tensor("dbg", [HPC, NG, 128, NJ, 128], f32,
                             kind="ExternalOutput")

    with tile.TileContext(nc) as tc:
        with (
            tc.tile_pool(name="qk", bufs=2) as qk_pool,
            tc.tile_pool(name="vp", bufs=2) as vp_pool,
            tc.tile_pool(name="ptb", bufs=8) as ptb_pool,
            tc.tile_pool(name="pti", bufs=8) as pti_pool,
            tc.tile_pool(name="wk", bufs=8) as wk_pool,
            tc.tile_pool(name="osb", bufs=2) as osb_pool,
            tc.tile_pool(name="rcp", bufs=16) as rcp_pool,
            tc.tile_pool(name="small", bufs=1) as small_pool,
            tc.tile_pool(name="psum_st", bufs=3, space="PSUM") as psum_st,
            tc.tile_pool(name="psum_o", bufs=2, space="PSUM") as psum_o,
        ):
            bias_t = small_pool.tile([128, 1], f32, tag="bias")
            nc.gpsimd.memset(bias_t, ACT_BIAS)
            # dummy exp so the ACT table set loads during the input DMAs
            warm_t = small_pool.tile([128, 1], f32, tag="warm")
            nc.scalar.activation(out=warm_t, in_=bias_t,
                                 func=mybir.ActivationFunctionType.Exp,
                                 bias=bias_t[:], scale=1.0)

            # ---- input loads; first head's first tiles load in small
            # chunks so the first QK starts as early as possible, and the
            # big transfers stay off the critical DMA path. ----
            kt_ts, qt_ts, vp_ts = {}, {}, {}
            for h in range(HPC):
                kt_ts[h] = qk_pool.tile([D + 1, S], f32r, tag="kt",
                                        name=f"kt_{h}")
                qt_ts[h] = qk_pool.tile([D + 1, S], f32r, tag="qt",
                                        name=f"qt_{h}")
                vp_ts[h] = vp_pool.tile([128, KT, D + 1], bf16, tag="vp",
                                        name=f"vp_{h}")
            nc.sync.dma_start(out=kt_ts[0][:, 0:256], in_=kT[0][:, 0:256])
            nc.scalar.dma_start(out=qt_ts[0][:, 0:512], in_=qT[0][:, 0:512])
            nc.sync.dma_start(out=kt_ts[0][:, 256:1024],
                              in_=kT[0][:, 256:1024])
            nc.sync.dma_start(out=kt_ts[0][:, 1024:2560],
                              in_=kT[0][:, 1024:2560])
            nc.sync.dma_start(out=kt_ts[0][:, 2560:S], in_=kT[0][:, 2560:S])
            nc.scalar.dma_start(
                out=vp_ts[0],
                in_=v[0].rearrange("(kt p) e -> p kt e", p=128))
            nc.sync.dma_start(out=qt_ts[0][:, 512:S], in_=qT[0][:, 512:S])
            nc.sync.dma_start(out=kt_ts[1], in_=kT[1])
            nc.sync.dma_start(out=qt_ts[1], in_=qT[1])
            nc.scalar.dma_start(
                out=vp_ts[1],
                in_=v[1].rearrange("(kt p) e -> p kt e", p=128))

            # ---- flat pipeline over (h, g, pair) ----
            steps = [(h, g, pp)
                     for h in range(HPC) for g in range(NG)
                     for pp in range(NPAIR)]
            pending = []          # (release_at_step, h, g, pp, pt_pair_ap)
            group_state = {}      # (h, g) -> dict(o, started, flushed)

            def get_group(h, g):
                key = (h, g)
                if key not in group_state:
                    group_state[key] = {
                        "o": psum_o.tile([128, NJ, 128], f32, tag="o",
                                         name=f"o_{h}_{g}"),
                        "bank_started": False,
                        "flushed": 0,
                    }
                return group_state[key]

            def emit_pv(h, g, pp, pt_b):
                gs = get_group(h, g)
                o_ps = gs["o"]
                for half in range(2):
                    kt = 2 * pp + half
                    last = (kt == KT - 1)
                    for j in range(NJ):
                        # start=True clears has_written for the WHOLE psum
                        # bank, so it is issued exactly once per o_ps tile;
                        # later writes to virgin elements overwrite, the
                        # rest accumulate.
                        nc.tensor.matmul(
                            o_ps[:, j, 0:D + 1],
                            lhsT=pt_b[:, half, j * 128:(j + 1) * 128],
                            rhs=vp_ts[h][:, kt, :],
                            start=(not gs["bank_started"]), stop=last,
                            skip_group_check=True,
                        )
                        gs["bank_started"] = True
                gs["flushed"] += 1
                if gs["flushed"] == NPAIR:
                    emit_norm(h, g)

            def emit_norm(h, g):
                gs = group_state[(h, g)]
                o_ps = gs["o"]
                out_sb = osb_pool.tile([128, NJ, D], f32, tag="osb",
                                       name=f"osb_{h}_{g}")
                out_view = out[h][g * GROUP:(g + 1) * GROUP, :].rearrange(
                    "(j p) d -> p j d", p=128)
                last_grp = (h == HPC - 1 and g == NG - 1)
                for j in range(NJ):
                    rcp_t = rcp_pool.tile([128, 1], f32, tag="rcp",
                                          name=f"rcp_{h}_{g}_{j}")
                    nc.vector.reciprocal(out=rcp_t, in_=o_ps[:, j, D:D + 1])
                    if last_grp and j % 2 == 1:
                        # tail: split muls across ACT/DVE, per-j DMAs on
                        # two queues so the last bytes leave ASAP
                        nc.scalar.activation(
                            out=out_sb[:, j, :], in_=o_ps[:, j, 0:D],
                            func=mybir.ActivationFunctionType.Copy,
                            scale=rcp_t[:])
                    else:
                        nc.vector.tensor_scalar_mul(
                            out_sb[:, j, :], o_ps[:, j, 0:D], rcp_t)
                    if last_grp:
                        q_eng = nc.sync if j % 2 == 0 else nc.scalar
                        q_eng.dma_start(out=out_view[:, j:j + 1, :],
                                        in_=out_sb[:, j:j + 1, :])
                if not last_grp:
                    nc.sync.dma_start(out=out_view, in_=out_sb)
                del group_state[(h, g)]

            for n, (h, g, pp) in enumerate(steps):
                q0 = g * GROUP
                st_t = psum_st.tile([128, 2, GROUP], f32, tag="st",
                                    name=f"st_{h}_{g}_{pp}")
                for half in range(2):
                    kt = 2 * pp + half
                    nc.tensor.matmul(
                        st_t[:, half, :],
                        lhsT=kt_ts[h][:, kt * 128:(kt + 1) * 128],
                        rhs=qt_ts[h][:, q0:q0 + GROUP],
                        start=True, stop=True,
                    )
                # exp over the whole pair [128, 2, GROUP]
                last_grp = (h == HPC - 1 and g == NG - 1)
                if last_grp:
                    pool_eps, dve_eps = POOL_EPS_LASTG, DVE_EPS_LASTG
                elif g % 2 == 0:
                    pool_eps, dve_eps = POOL_EPS_A, DVE_EPS_A
                else:
                    pool_eps, dve_eps = POOL_EPS_B, DVE_EPS_B
                if pp in pool_eps or pp in dve_eps:
                    w_t = wk_pool.tile([128, 2, GROUP], f32, tag="wk")
                    nc.vector._custom_dve(
                        op_expbits, out=w_t, in0=st_t,
                        s0=C0_MAGIC, s1=C1_QUAD, imm2=C2_QUAD)
                    pt_i = pti_pool.tile([128, 2, GROUP], i16, tag="pti")
                    if pp in dve_eps:
                        nc.vector.tensor_scalar_add(pt_i, w_t, K_BITS)
                    else:
                        nc.gpsimd.tensor_scalar_add(pt_i, w_t, K_BITS)
                    pt_b = pt_i.bitcast(mybir.dt.bfloat16)
                    lag = LAG_DVE
                else:
                    pt_bt = ptb_pool.tile([128, 2, GROUP],
                                          mybir.dt.bfloat16, tag="ptb")
                    nc.scalar.activation(
                        out=pt_bt, in_=st_t,
                        func=mybir.ActivationFunctionType.Exp,
                        bias=bias_t[:], scale=ACT_SCALE)
                    pt_b = pt_bt
                    lag = LAG_ACT
                if last_grp and pp >= NPAIR - 3:
                    lag = 1  # shorten the kernel tail (these pairs are ACT)
                pending.append((n + lag, h, g, pp, pt_b))
                while pending and pending[0][0] <= n:
                    _, fh, fg, fpp, fpt = pending.pop(0)
                    emit_pv(fh, fg, fpp, fpt)

            while pending:
                _, fh, fg, fpp, fpt = pending.pop(0)
                emit_pv(fh, fg, fpp, fpt)

    nc.compile()
    return nc


def _get_compiled():
    global _compiled
    if _compiled is None:
        _compiled = _build()
    return _compiled


def _to_bf16(x):
    b = np.ascontiguousarray(x, np.float32).view(np.uint32)
    r = ((b >> 16) + ((b >> 15) & 1)).astype(np.uint16)
    return r


def kernel(query: np.ndarray, key: np.ndarray, value: np.ndarray) -> np.ndarray:
    import ml_dtypes
    from concourse.bass_utils import run_bass_kernel_spmd

    nc = _get_compiled()

    q = np.asarray(query, dtype=np.float32).reshape(H, S, D)
    k = np.asarray(key, dtype=np.float32).reshape(H, S, D)
    v = np.asarray(value, dtype=np.float32).reshape(H, S, D)

    ksc = np.float32(128.0 * LOG2E)
    in_maps = []
    for c in range(N_CORES):
        hs = slice(c * HPC, (c + 1) * HPC)
        qh = q[hs].transpose(0, 2, 1)                       # [HPC, 64, S]
        kh = k[hs].transpose(0, 2, 1) * ksc                 # [HPC, 64, S]
        qT_host = np.concatenate(
            [qh, np.full((HPC, 1, S), SROW, np.float32)], axis=1)
        kT_host = np.concatenate(
            [kh, np.ones((HPC, 1, S), np.float32)], axis=1)
        v_host = np.concatenate(
            [v[hs], np.ones((HPC, S, 1), np.float32)], axis=-1)
        in_maps.append({
            "qT": np.ascontiguousarray(qT_host),
            "kT": np.ascontiguousarray(kT_host),
            "v": _to_bf16(v_host).view(ml_dtypes.bfloat16),
        })

    res = run_bass_kernel_spmd(nc, in_maps, list(range(N_CORES)))

    outp = np.empty((B, H, S, D), dtype=np.float32)
    for c in range(N_CORES):
        for hh in range(HPC):
            outp[0, c * HPC + hh] = res.results[c]["out"][hh]
    return outp


# revision 30
# speedup vs baseline: 1.0930x; 1.0848x over previous
"""Dense dot-product attention (B=1, H=16, S=4096, D=64, fp32) on 8 trn2 cores.

Head-parallel: core c computes heads [2c, 2c+1] fully on-device, no comms.

Per-head algorithm (S^T score layout, [q,d] output layout):
  scores: S^T[k, q] PSUM tile per (k-tile, q-group) via f32r matmuls with a
    65-row contraction: rows 0:64 contract d (K pre-scaled by 128*log2e),
    row 64 is a bias row adding SROW = 128*(127 - SH) - 64, so the PSUM
    value is v = 128*(x + 127) - 64 with x = score*log2e - SH.
  exp -> P (bf16) via two engine paths, split per k-tile:
    ACT path: P = Exp(v * ln2/128 + bias), full precision, bf16 out.
    DVE+Pool path ("expbits"): DVE custom op computes
        w = (C1*_t + C2)*_t + v,  _t = v - 128*round(v/128)  (magic round)
      then Pool adds K and converts to int16; the int16 IS the bf16 bit
      pattern of 2^x (exponent from the 128-quantized part, mantissa from
      the quadratic 2^frac fit; max weight err ~1%, rel out err ~3e-3).
  PV: out[q, d'] accumulates over 32 k-tiles via bf16 matmuls with
    lhsT = P^T[k, q-subtile] (ap=65/matmul), rhs = V'[k, 0:65] where V' has
    a ones column so out[q, 64] = sum of P (softmax denominator).
  normalize: per q-subtile reciprocal + per-partition scalar multiply;
    output written in natural [q, d] layout (no transposes).

PV matmuls are emitted LAG k-tiles behind QK so the exp chain latency never
stalls the in-order PE queue.
"""

import sys

if "/opt/trn_rl_repo" not in sys.path:
    sys.path.insert(0, "/opt/trn_rl_repo")

import numpy as np

B, H, S, D = 1, 16, 4096, 64
N_CORES = 8
HPC = H // N_CORES    # heads per core = 2

KT = S // 128         # 32 k-tiles per head
NPAIR = KT // 2       # k-tiles are processed in pairs of 2
GROUP = 512           # q columns per score group
NG = S // GROUP       # 8 groups per head
NJ = GROUP // 128     # 4 q-subtiles per group

LOG2E = 1.4426950408889634
LN2 = 0.6931471805599453
SH = 46.0                                   # shift in log2 units
SROW = 128.0 * (127.0 - SH) - 64.0          # bias-row constant (J = -64)
ACT_SCALE = LN2 / 128.0
ACT_BIAS = -(127.0 - 64.0 / 128.0) * LN2    # undoes 128*( . +127) - 64

# expbits constants: v + quad(_t) + K == bf16 bits of 2^x (see numcheck)
C0_MAGIC = 1.5 * 2**30
C1_QUAD = 2.459070897941e-03
C2_QUAD = -1.995185412854e-02
K_BITS = 53.044930589134

# k-tile PAIRS handled by the expbits path (rest go to ACT). Pool's
# 1517ns/pair op can only sustain one every 3 pairs, so the Pool-finished
# pairs are spaced 3 apart (11 per 2 groups); 2 more pairs per 2 groups are
# DVE-finished. Every engine ends up <=95% of the PE-bound group period.
POOL_EPS_A = frozenset({2, 5, 8, 11, 14})      # even groups, Pool-finished
POOL_EPS_B = frozenset({1, 4, 7, 10, 13, 15})  # odd groups, Pool-finished
DVE_EPS_A = frozenset({0})                     # DVE-finished
DVE_EPS_B = frozenset({8})
POOL_EPS_LASTG = frozenset({1, 4, 7, 10})      # final group: early only
DVE_EPS_LASTG = frozenset({12})
LAG_ACT = 4   # in pair units
LAG_DVE = 9
DEBUG_DUMP = False

_compiled = None


def _register_expbits_op():
    import concourse.dve_ops as dve_ops
    from concourse.dve_ops import DveOp, OPS, has_src1
    from concourse.dve_spec import Spec, Src0, C0, C1, C2, lower
    from concourse.dve_uop import DveOpSpec

    if "EXPBITS_ANT" in dve_ops._SUB_OPCODE_FOR_NAME:
        return {op.name: op for op in OPS}["EXPBITS_ANT"]

    from concourse.dve_spec import Zero, maxx

    f32 = np.float32

    def ref(in0, in1, s0, s1, imm2):
        x = in0.astype(np.float32)
        r = x + f32(s0)
        s = r - f32(s0)
        t = x - s
        return np.maximum((t * f32(s1) + f32(imm2)) * t + x, f32(0.0))

    _r = Src0 + C0
    _t = Src0 - (_r - C0)
    op = DveOp(
        "EXPBITS_ANT",
        # max(.., 0): deeply negative scores (x < -127) would otherwise go
        # negative in int16 and bitcast to huge negative bf16 weights.
        Spec(body=maxx((_t * C1 + C2) * _t + Src0, Zero), reference=ref),
        subdim=False,
        uops_sha={},
    )
    OPS.append(op)
    dve_ops.CUSTOM_DVE_SPECS[op.name] = op.spec
    dve_ops._SUB_OPCODE_FOR_NAME[op.name] = (
        dve_ops._CUSTOM_DVE_ROW_BASE + len(dve_ops._SUB_OPCODE_FOR_NAME))
    for ver in ("v3", "v4"):
        try:
            compiled = DveOpSpec(
                name=op.name,
                opcode=dve_ops._SUB_OPCODE_FOR_NAME[op.name],
                uops=lower(op.spec, ver=ver),
                rd1_en=has_src1(op.spec),
            )
            op.uops_sha[ver] = compiled.sha(ver)
        except Exception:
            pass
    return op


def _build():
    import concourse.bacc as bacc
    import concourse.mybir as mybir
    import concourse.tile as tile

    op_expbits = _register_expbits_op()

    f32 = mybir.dt.float32
    f32r = mybir.dt.float32r
    bf16 = mybir.dt.bfloat16
    i16 = mybir.dt.int16

    nc = bacc.Bacc("TRN2", target_bir_lowering=False, debug=False,
                   num_devices=N_CORES)

    qT = nc.dram_tensor("qT", [HPC, D + 1, S], f32r, kind="ExternalInput")
    kT = nc.dram_tensor("kT", [HPC, D + 1, S], f32r, kind="ExternalInput")
    v = nc.dram_tensor("v", [HPC, S, D + 1], bf16, kind="ExternalInput")
    out = nc.dram_tensor("out", [HPC, S, D], f32, kind="ExternalOutput")
    dbg = None
    if DEBUG_DUMP:
        dbg = nc.dram_tensor("dbg", [HPC, NG, 128, NJ, 128], f32,
                             kind="ExternalOutput")

    with tile.TileContext(nc) as tc:
        with (
            tc.tile_pool(name="qk", bufs=2) as qk_pool,
            tc.tile_pool(name="vp", bufs=2) as vp_pool,
            tc.tile_pool(name="ptb", bufs=8) as ptb_pool,
            tc.tile_pool(name="pti", bufs=8) as pti_pool,
            tc.tile_pool(name="wk", bufs=8) as wk_pool,
            tc.tile_pool(name="osb", bufs=2) as osb_pool,
            tc.tile_pool(name="rcp", bufs=16) as rcp_pool,
            tc.tile_pool(name="small", bufs=1) as small_pool,
            tc.tile_pool(name="psum_st", bufs=3, space="PSUM") as psum_st,
            tc.tile_pool(name="psum_o", bufs=2, space="PSUM") as psum_o,
        ):
            bias_t = small_pool.tile([128, 1], f32, tag="bias")
            nc.gpsimd.memset(bias_t, ACT_BIAS)
            # dummy exp so the ACT table set loads during the input DMAs
            warm_t = small_pool.tile([128, 1], f32, tag="warm")
            nc.scalar.activation(out=warm_t, in_=bias_t,
                                 func=mybir.ActivationFunctionType.Exp,
                                 bias=bias_t[:], scale=1.0)

            # ---- input loads; first head's first tiles load in small
            # chunks so the first QK starts as early as possible, and the
            # big transfers stay off the critical DMA path. ----
            kt_ts, qt_ts, vp_ts = {}, {}, {}
            for h in range(HPC):
                kt_ts[h] = qk_pool.tile([D + 1, S], f32r, tag="kt",
                                        name=f"kt_{h}")
                qt_ts[h] = qk_pool.tile([D + 1, S], f32r, tag="qt",
                                        name=f"qt_{h}")
                vp_ts[h] = vp_pool.tile([128, KT, D + 1], bf16, tag="vp",
                                        name=f"vp_{h}")
            nc.sync.dma_start(out=kt_ts[0][:, 0:256], in_=kT[0][:, 0:256])
            nc.scalar.dma_start(out=qt_ts[0][:, 0:512], in_=qT[0][:, 0:512])
            nc.sync.dma_start(out=kt_ts[0][:, 256:1024],
                              in_=kT[0][:, 256:1024])
            nc.sync.dma_start(out=kt_ts[0][:, 1024:2560],
                              in_=kT[0][:, 1024:2560])
            nc.sync.dma_start(out=kt_ts[0][:, 2560:S], in_=kT[0][:, 2560:S])
            nc.scalar.dma_start(
                out=vp_ts[0],
                in_=v[0].rearrange("(kt p) e -> p kt e", p=128))
            nc.sync.dma_start(out=qt_ts[0][:, 512:S], in_=qT[0][:, 512:S])
            nc.sync.dma_start(out=kt_ts[1], in_=kT[1])
            nc.sync.dma_start(out=qt_ts[1], in_=qT[1])
            nc.scalar.dma_start(
                out=vp_ts[1],
                in_=v[1].rearrange("(kt p) e -> p kt e", p=128))

            # ---- flat pipeline over (h, g, pair) ----
            steps = [(h, g, pp)
                     for h in range(HPC) for g in range(NG)
                     for pp in range(NPAIR)]
            pending = []          # (release_at_step, h, g, pp, pt_pair_ap)
            group_state = {}      # (h, g) -> dict(o, started, flushed)

            def get_group(h, g):
                key = (h, g)
                if key not in group_state:
                    group_state[key] = {
                        "o": psum_o.tile([128, NJ, 128], f32, tag="o",
                                         name=f"o_{h}_{g}"),
                        "bank_started": False,
                        "flushed": 0,
                    }
                return group_state[key]

            def emit_pv(h, g, pp, pt_b):
                gs = get_group(h, g)
                o_ps = gs["o"]
                gs["flushed"] += 1
                # PV pairs may be emitted out of kt order (priority flush);
                # stop goes on the last EMITTED pair, which closes every j
                # region since every pair writes all of them.
                last = (gs["flushed"] == NPAIR)
                for half in range(2):
                    kt = 2 * pp + half
                    for j in range(NJ):
                        # start=True clears has_written for the WHOLE psum
                        # bank, so it is issued exactly once per o_ps tile;
                        # later writes to virgin elements overwrite, the
                        # rest accumulate.
                        nc.tensor.matmul(
                            o_ps[:, j, 0:D + 1],
                            lhsT=pt_b[:, half, j * 128:(j + 1) * 128],
                            rhs=vp_ts[h][:, kt, :],
                            start=(not gs["bank_started"]),
                            stop=(last and half == 1),
                            skip_group_check=True,
                        )
                        gs["bank_started"] = True
                if last:
                    emit_norm(h, g)

            def emit_norm(h, g):
                gs = group_state[(h, g)]
                o_ps = gs["o"]
                out_sb = osb_pool.tile([128, NJ, D], f32, tag="osb",
                                       name=f"osb_{h}_{g}")
                out_view = out[h][g * GROUP:(g + 1) * GROUP, :].rearrange(
                    "(j p) d -> p j d", p=128)
                last_grp = (h == HPC - 1 and g == NG - 1)
                for j in range(NJ):
                    rcp_t = rcp_pool.tile([128, 1], f32, tag="rcp",
                                          name=f"rcp_{h}_{g}_{j}")
                    nc.vector.reciprocal(out=rcp_t, in_=o_ps[:, j, D:D + 1])
                    if last_grp and j % 2 == 1:
                        # tail: split muls across ACT/DVE, per-j DMAs on
                        # two queues so the last bytes leave ASAP
                        nc.scalar.activation(
                            out=out_sb[:, j, :], in_=o_ps[:, j, 0:D],
                            func=mybir.ActivationFunctionType.Copy,
                            scale=rcp_t[:])
                    else:
                        nc.vector.tensor_scalar_mul(
                            out_sb[:, j, :], o_ps[:, j, 0:D], rcp_t)
                    if last_grp:
                        q_eng = nc.sync if j % 2 == 0 else nc.scalar
                        q_eng.dma_start(out=out_view[:, j:j + 1, :],
                                        in_=out_sb[:, j:j + 1, :])
                if not last_grp:
                    nc.sync.dma_start(out=out_view, in_=out_sb)
                del group_state[(h, g)]

            for n, (h, g, pp) in enumerate(steps):
                q0 = g * GROUP
                st_t = psum_st.tile([128, 2, GROUP], f32, tag="st",
                                    name=f"st_{h}_{g}_{pp}")
                for half in range(2):
                    kt = 2 * pp + half
                    nc.tensor.matmul(
                        st_t[:, half, :],
                        lhsT=kt_ts[h][:, kt * 128:(kt + 1) * 128],
                        rhs=qt_ts[h][:, q0:q0 + GROUP],
                        start=True, stop=True,
                    )
                # exp over the whole pair [128, 2, GROUP]
                last_grp = (h == HPC - 1 and g == NG - 1)
                if last_grp:
                    pool_eps, dve_eps = POOL_EPS_LASTG, DVE_EPS_LASTG
                elif g % 2 == 0:
                    pool_eps, dve_eps = POOL_EPS_A, DVE_EPS_A
                else:
                    pool_eps, dve_eps = POOL_EPS_B, DVE_EPS_B
                if pp in pool_eps or pp in dve_eps:
                    w_t = wk_pool.tile([128, 2, GROUP], f32, tag="wk")
                    nc.vector._custom_dve(
                        op_expbits, out=w_t, in0=st_t,
                        s0=C0_MAGIC, s1=C1_QUAD, imm2=C2_QUAD)
                    pt_i = pti_pool.tile([128, 2, GROUP], i16, tag="pti")
                    if pp in dve_eps:
                        nc.vector.tensor_scalar_add(pt_i, w_t, K_BITS)
                    else:
                        nc.gpsimd.tensor_scalar_add(pt_i, w_t, K_BITS)
                    pt_b = pt_i.bitcast(mybir.dt.bfloat16)
                    lag = LAG_DVE
                else:
                    pt_bt = ptb_pool.tile([128, 2, GROUP],
                                          mybir.dt.bfloat16, tag="ptb")
                    nc.scalar.activation(
                        out=pt_bt, in_=st_t,
                        func=mybir.ActivationFunctionType.Exp,
                        bias=bias_t[:], scale=ACT_SCALE)
                    pt_b = pt_bt
                    lag = LAG_ACT
                if last_grp and pp >= NPAIR - 3:
                    lag = 1  # shorten the kernel tail (these pairs are ACT)
                pending.append((n + lag, h, g, pp, pt_b))
                ready = [e for e in pending if e[0] <= n]
                if ready:
                    pending = [e for e in pending if e[0] > n]
                    for _, fh, fg, fpp, fpt in ready:
                        emit_pv(fh, fg, fpp, fpt)

            for _, fh, fg, fpp, fpt in pending:
                emit_pv(fh, fg, fpp, fpt)

    nc.compile()
    return nc


def _get_compiled():
    global _compiled
    if _compiled is None:
        _compiled = _build()
    return _compiled


def _to_bf16(x):
    b = np.ascontiguousarray(x, np.float32).view(np.uint32)
    r = ((b >> 16) + ((b >> 15) & 1)).astype(np.uint16)
    return r


def kernel(query: np.ndarray, key: np.ndarray, value: np.ndarray) -> np.ndarray:
    import ml_dtypes
    from concourse.bass_utils import run_bass_kernel_spmd

    nc = _get_compiled()

    q = np.asarray(query, dtype=np.float32).reshape(H, S, D)
    k = np.asarray(key, dtype=np.float32).reshape(H, S, D)
    v = np.asarray(value, dtype=np.float32).reshape(H, S, D)

    ksc = np.float32(128.0 * LOG2E)
    in_maps = []
    for c in range(N_CORES):
        hs = slice(c * HPC, (c + 1) * HPC)
        qh = q[hs].transpose(0, 2, 1)                       # [HPC, 64, S]
        kh = k[hs].transpose(0, 2, 1) * ksc                 # [HPC, 64, S]
        qT_host = np.concatenate(
            [qh, np.full((HPC, 1, S), SROW, np.float32)], axis=1)
        kT_host = np.concatenate(
            [kh, np.ones((HPC, 1, S), np.float32)], axis=1)
        v_host = np.concatenate(
            [v[hs], np.ones((HPC, S, 1), np.float32)], axis=-1)
        in_maps.append({
            "qT": np.ascontiguousarray(qT_host),
            "kT": np.ascontiguousarray(kT_host),
            "v": _to_bf16(v_host).view(ml_dtypes.bfloat16),
        })

    res = run_bass_kernel_spmd(nc, in_maps, list(range(N_CORES)))

    outp = np.empty((B, H, S, D), dtype=np.float32)
    for c in range(N_CORES):
        for hh in range(HPC):
            outp[0, c * HPC + hh] = res.results[c]["out"][hh]
    return outp


# revision 33
# speedup vs baseline: 1.1015x; 1.0077x over previous
"""Dense dot-product attention (B=1, H=16, S=4096, D=64, fp32) on 8 trn2 cores.

Head-parallel: core c computes heads [2c, 2c+1] fully on-device, no comms.

Per-head algorithm (S^T score layout, [q,d] output layout):
  scores: S^T[k, q] PSUM tile per (k-tile, q-group) via f32r matmuls with a
    65-row contraction: rows 0:64 contract d (K pre-scaled by 128*log2e),
    row 64 is a bias row adding SROW = 128*(127 - SH) - 64, so the PSUM
    value is v = 128*(x + 127) - 64 with x = score*log2e - SH.
  exp -> P (bf16) via two engine paths, split per k-tile:
    ACT path: P = Exp(v * ln2/128 + bias), full precision, bf16 out.
    DVE+Pool path ("expbits"): DVE custom op computes
        w = (C1*_t + C2)*_t + v,  _t = v - 128*round(v/128)  (magic round)
      then Pool adds K and converts to int16; the int16 IS the bf16 bit
      pattern of 2^x (exponent from the 128-quantized part, mantissa from
      the quadratic 2^frac fit; max weight err ~1%, rel out err ~3e-3).
  PV: out[q, d'] accumulates over 32 k-tiles via bf16 matmuls with
    lhsT = P^T[k, q-subtile] (ap=65/matmul), rhs = V'[k, 0:65] where V' has
    a ones column so out[q, 64] = sum of P (softmax denominator).
  normalize: per q-subtile reciprocal + per-partition scalar multiply;
    output written in natural [q, d] layout (no transposes).

PV matmuls are emitted LAG k-tiles behind QK so the exp chain latency never
stalls the in-order PE queue.
"""

import sys

if "/opt/trn_rl_repo" not in sys.path:
    sys.path.insert(0, "/opt/trn_rl_repo")

import numpy as np

B, H, S, D = 1, 16, 4096, 64
N_CORES = 8
HPC = H // N_CORES    # heads per core = 2

KT = S // 128         # 32 k-tiles per head
NPAIR = KT // 2       # k-tiles are processed in pairs of 2
GROUP = 512           # q columns per score group
NG = S // GROUP       # 8 groups per head
NJ = GROUP // 128     # 4 q-subtiles per group

LOG2E = 1.4426950408889634
LN2 = 0.6931471805599453
SH = 46.0                                   # shift in log2 units
SROW = 128.0 * (127.0 - SH) - 64.0          # bias-row constant (J = -64)
ACT_SCALE = LN2 / 128.0
ACT_BIAS = -(127.0 - 64.0 / 128.0) * LN2    # undoes 128*( . +127) - 64

# expbits constants: v + quad(_t) + K == bf16 bits of 2^x (see numcheck)
C0_MAGIC = 1.5 * 2**30
C1_QUAD = 2.459070897941e-03
C2_QUAD = -1.995185412854e-02
K_BITS = 53.044930589134

# k-tile PAIRS handled by the expbits path (rest go to ACT). Pool's
# 1517ns/pair op can only sustain one every 3 pairs, so the Pool-finished
# pairs are spaced 3 apart (11 per 2 groups); 2 more pairs per 2 groups are
# DVE-finished. Every engine ends up <=95% of the PE-bound group period.
POOL_EPS_A = frozenset({2, 5, 8, 11, 14})      # even groups, Pool-finished
POOL_EPS_B = frozenset({1, 4, 7, 10, 13, 15})  # odd groups, Pool-finished
DVE_EPS_A = frozenset({0})                     # DVE-finished
DVE_EPS_B = frozenset({8})
POOL_EPS_LASTG = frozenset({1, 4, 7, 10})      # final group: early only
DVE_EPS_LASTG = frozenset({12})
LAG_ACT = 4   # in pair units
LAG_DVE = 9
DEBUG_DUMP = False

_compiled = None


def _register_expbits_op():
    import concourse.dve_ops as dve_ops
    from concourse.dve_ops import DveOp, OPS, has_src1
    from concourse.dve_spec import Spec, Src0, C0, C1, C2, lower
    from concourse.dve_uop import DveOpSpec

    if "EXPBITS_ANT" in dve_ops._SUB_OPCODE_FOR_NAME:
        return {op.name: op for op in OPS}["EXPBITS_ANT"]

    from concourse.dve_spec import Zero, maxx

    f32 = np.float32

    def ref(in0, in1, s0, s1, imm2):
        x = in0.astype(np.float32)
        r = x + f32(s0)
        s = r - f32(s0)
        t = x - s
        return np.maximum((t * f32(s1) + f32(imm2)) * t + x, f32(0.0))

    _r = Src0 + C0
    _t = Src0 - (_r - C0)
    op = DveOp(
        "EXPBITS_ANT",
        # max(.., 0): deeply negative scores (x < -127) would otherwise go
        # negative in int16 and bitcast to huge negative bf16 weights.
        Spec(body=maxx((_t * C1 + C2) * _t + Src0, Zero), reference=ref),
        subdim=False,
        uops_sha={},
    )
    OPS.append(op)
    dve_ops.CUSTOM_DVE_SPECS[op.name] = op.spec
    dve_ops._SUB_OPCODE_FOR_NAME[op.name] = (
        dve_ops._CUSTOM_DVE_ROW_BASE + len(dve_ops._SUB_OPCODE_FOR_NAME))
    for ver in ("v3", "v4"):
        try:
            compiled = DveOpSpec(
                name=op.name,
                opcode=dve_ops._SUB_OPCODE_FOR_NAME[op.name],
                uops=lower(op.spec, ver=ver),
                rd1_en=has_src1(op.spec),
            )
            op.uops_sha[ver] = compiled.sha(ver)
        except Exception:
            pass
    return op


def _build():
    import concourse.bacc as bacc
    import concourse.mybir as mybir
    import concourse.tile as tile

    op_expbits = _register_expbits_op()

    f32 = mybir.dt.float32
    f32r = mybir.dt.float32r
    bf16 = mybir.dt.bfloat16
    i16 = mybir.dt.int16

    nc = bacc.Bacc("TRN2", target_bir_lowering=False, debug=False,
                   num_devices=N_CORES)

    qT = nc.dram_tensor("qT", [HPC, D + 1, S], f32r, kind="ExternalInput")
    kT = nc.dram_tensor("kT", [HPC, D + 1, S], f32r, kind="ExternalInput")
    v = nc.dram_tensor("v", [HPC, S, D + 1], bf16, kind="ExternalInput")
    out = nc.dram_tensor("out", [HPC, S, D], f32, kind="ExternalOutput")
    dbg = None
    if DEBUG_DUMP:
        dbg = nc.dram_tensor("dbg", [HPC, NG, 128, NJ, 128], f32,
                             kind="ExternalOutput")

    with tile.TileContext(nc) as tc:
        with (
            tc.tile_pool(name="qk", bufs=2) as qk_pool,
            tc.tile_pool(name="vp", bufs=2) as vp_pool,
            tc.tile_pool(name="ptb", bufs=8) as ptb_pool,
            tc.tile_pool(name="pti", bufs=8) as pti_pool,
            tc.tile_pool(name="wk", bufs=8) as wk_pool,
            tc.tile_pool(name="osb", bufs=2) as osb_pool,
            tc.tile_pool(name="rcp", bufs=16) as rcp_pool,
            tc.tile_pool(name="small", bufs=1) as small_pool,
            tc.tile_pool(name="psum_st", bufs=3, space="PSUM") as psum_st,
            tc.tile_pool(name="psum_o", bufs=2, space="PSUM") as psum_o,
        ):
            bias_t = small_pool.tile([128, 1], f32, tag="bias")
            nc.gpsimd.memset(bias_t, ACT_BIAS)
            # dummy exp so the ACT table set loads during the input DMAs
            warm_t = small_pool.tile([128, 1], f32, tag="warm")
            nc.scalar.activation(out=warm_t, in_=bias_t,
                                 func=mybir.ActivationFunctionType.Exp,
                                 bias=bias_t[:], scale=1.0)

            # ---- input loads; first head's first tiles load in small
            # chunks so the first QK starts as early as possible, and the
            # big transfers stay off the critical DMA path. ----
            kt_ts, qt_ts, vp_ts = {}, {}, {}
            for h in range(HPC):
                kt_ts[h] = qk_pool.tile([D + 1, S], f32r, tag="kt",
                                        name=f"kt_{h}")
                qt_ts[h] = qk_pool.tile([D + 1, S], f32r, tag="qt",
                                        name=f"qt_{h}")
                vp_ts[h] = vp_pool.tile([128, KT, D + 1], bf16, tag="vp",
                                        name=f"vp_{h}")
            v0_view = v[0].rearrange("(kt p) e -> p kt e", p=128)
            nc.sync.dma_start(out=kt_ts[0][:, 0:256], in_=kT[0][:, 0:256])
            nc.scalar.dma_start(out=qt_ts[0][:, 0:512], in_=qT[0][:, 0:512])
            nc.sync.dma_start(out=kt_ts[0][:, 256:1024],
                              in_=kT[0][:, 256:1024])
            nc.scalar.dma_start(out=vp_ts[0][:, 0:8, :], in_=v0_view[:, 0:8, :])
            nc.sync.dma_start(out=kt_ts[0][:, 1024:2560],
                              in_=kT[0][:, 1024:2560])
            nc.scalar.dma_start(out=vp_ts[0][:, 8:KT, :],
                                in_=v0_view[:, 8:KT, :])
            nc.sync.dma_start(out=kt_ts[0][:, 2560:S], in_=kT[0][:, 2560:S])
            nc.sync.dma_start(out=qt_ts[0][:, 512:S], in_=qT[0][:, 512:S])
            nc.sync.dma_start(out=kt_ts[1], in_=kT[1])
            nc.sync.dma_start(out=qt_ts[1], in_=qT[1])
            nc.scalar.dma_start(
                out=vp_ts[1],
                in_=v[1].rearrange("(kt p) e -> p kt e", p=128))

            # ---- flat pipeline over (h, g, pair) ----
            steps = [(h, g, pp)
                     for h in range(HPC) for g in range(NG)
                     for pp in range(NPAIR)]
            pending = []          # (release_at_step, h, g, pp, pt_pair_ap)
            group_state = {}      # (h, g) -> dict(o, started, flushed)

            def get_group(h, g):
                key = (h, g)
                if key not in group_state:
                    group_state[key] = {
                        "o": psum_o.tile([128, NJ, 128], f32, tag="o",
                                         name=f"o_{h}_{g}"),
                        "bank_started": False,
                        "flushed": 0,
                    }
                return group_state[key]

            def emit_pv(h, g, pp, pt_b):
                gs = get_group(h, g)
                o_ps = gs["o"]
                gs["flushed"] += 1
                # PV pairs may be emitted out of kt order (priority flush);
                # stop goes on the last EMITTED pair, which closes every j
                # region since every pair writes all of them.
                last = (gs["flushed"] == NPAIR)
                for half in range(2):
                    kt = 2 * pp + half
                    for j in range(NJ):
                        # start=True clears has_written for the WHOLE psum
                        # bank, so it is issued exactly once per o_ps tile;
                        # later writes to virgin elements overwrite, the
                        # rest accumulate.
                        nc.tensor.matmul(
                            o_ps[:, j, 0:D + 1],
                            lhsT=pt_b[:, half, j * 128:(j + 1) * 128],
                            rhs=vp_ts[h][:, kt, :],
                            start=(not gs["bank_started"]),
                            stop=(last and half == 1),
                            skip_group_check=True,
                        )
                        gs["bank_started"] = True
                if last:
                    emit_norm(h, g)

            def emit_norm(h, g):
                gs = group_state[(h, g)]
                o_ps = gs["o"]
                out_sb = osb_pool.tile([128, NJ, D], f32, tag="osb",
                                       name=f"osb_{h}_{g}")
                out_view = out[h][g * GROUP:(g + 1) * GROUP, :].rearrange(
                    "(j p) d -> p j d", p=128)
                last_grp = (h == HPC - 1 and g == NG - 1)
                for j in range(NJ):
                    rcp_t = rcp_pool.tile([128, 1], f32, tag="rcp",
                                          name=f"rcp_{h}_{g}_{j}")
                    nc.vector.reciprocal(out=rcp_t, in_=o_ps[:, j, D:D + 1])
                    if last_grp and j % 2 == 1:
                        # tail: split muls across ACT/DVE, per-j DMAs on
                        # two queues so the last bytes leave ASAP
                        nc.scalar.activation(
                            out=out_sb[:, j, :], in_=o_ps[:, j, 0:D],
                            func=mybir.ActivationFunctionType.Copy,
                            scale=rcp_t[:])
                    else:
                        nc.vector.tensor_scalar_mul(
                            out_sb[:, j, :], o_ps[:, j, 0:D], rcp_t)
                    if last_grp:
                        q_eng = nc.sync if j % 2 == 0 else nc.scalar
                        q_eng.dma_start(out=out_view[:, j:j + 1, :],
                                        in_=out_sb[:, j:j + 1, :])
                if not last_grp:
                    nc.sync.dma_start(out=out_view, in_=out_sb)
                del group_state[(h, g)]

            for n, (h, g, pp) in enumerate(steps):
                q0 = g * GROUP
                st_t = psum_st.tile([128, 2, GROUP], f32, tag="st",
                                    name=f"st_{h}_{g}_{pp}")
                for half in range(2):
                    kt = 2 * pp + half
                    nc.tensor.matmul(
                        st_t[:, half, :],
                        lhsT=kt_ts[h][:, kt * 128:(kt + 1) * 128],
                        rhs=qt_ts[h][:, q0:q0 + GROUP],
                        start=True, stop=True,
                    )
                # exp over the whole pair [128, 2, GROUP]
                last_grp = (h == HPC - 1 and g == NG - 1)
                if last_grp:
                    pool_eps, dve_eps = POOL_EPS_LASTG, DVE_EPS_LASTG
                elif g % 2 == 0:
                    pool_eps, dve_eps = POOL_EPS_A, DVE_EPS_A
                else:
                    pool_eps, dve_eps = POOL_EPS_B, DVE_EPS_B
                if pp in pool_eps or pp in dve_eps:
                    w_t = wk_pool.tile([128, 2, GROUP], f32, tag="wk")
                    nc.vector._custom_dve(
                        op_expbits, out=w_t, in0=st_t,
                        s0=C0_MAGIC, s1=C1_QUAD, imm2=C2_QUAD)
                    pt_i = pti_pool.tile([128, 2, GROUP], i16, tag="pti")
                    if pp in dve_eps:
                        nc.vector.tensor_scalar_add(pt_i, w_t, K_BITS)
                    else:
                        nc.gpsimd.tensor_scalar_add(pt_i, w_t, K_BITS)
                    pt_b = pt_i.bitcast(mybir.dt.bfloat16)
                    lag = LAG_DVE
                else:
                    pt_bt = ptb_pool.tile([128, 2, GROUP],
                                          mybir.dt.bfloat16, tag="ptb")
                    nc.scalar.activation(
                        out=pt_bt, in_=st_t,
                        func=mybir.ActivationFunctionType.Exp,
                        bias=bias_t[:], scale=ACT_SCALE)
                    pt_b = pt_bt
                    lag = LAG_ACT
                if last_grp and pp >= NPAIR - 3:
                    lag = 1  # shorten the kernel tail (these pairs are ACT)
                pending.append((n + lag, h, g, pp, pt_b))
                ready = [e for e in pending if e[0] <= n]
                if ready:
                    pending = [e for e in pending if e[0] > n]
                    for _, fh, fg, fpp, fpt in ready:
                        emit_pv(fh, fg, fpp, fpt)

            for _, fh, fg, fpp, fpt in pending:
                emit_pv(fh, fg, fpp, fpt)

    nc.compile()
    return nc


def _get_compiled():
    global _compiled
    if _compiled is None:
        _compiled = _build()
    return _compiled


def _to_bf16(x):
    b = np.ascontiguousarray(x, np.float32).view(np.uint32)
    r = ((b >> 16) + ((b >> 15) & 1)).astype(np.uint16)
    return r


def kernel(query: np.ndarray, key: np.ndarray, value: np.ndarray) -> np.ndarray:
    import ml_dtypes
    from concourse.bass_utils import run_bass_kernel_spmd

    nc = _get_compiled()

    q = np.asarray(query, dtype=np.float32).reshape(H, S, D)
    k = np.asarray(key, dtype=np.float32).reshape(H, S, D)
    v = np.asarray(value, dtype=np.float32).reshape(H, S, D)

    ksc = np.float32(128.0 * LOG2E)
    in_maps = []
    for c in range(N_CORES):
        hs = slice(c * HPC, (c + 1) * HPC)
        qh = q[hs].transpose(0, 2, 1)                       # [HPC, 64, S]
        kh = k[hs].transpose(0, 2, 1) * ksc                 # [HPC, 64, S]
        qT_host = np.concatenate(
            [qh, np.full((HPC, 1, S), SROW, np.float32)], axis=1)
        kT_host = np.concatenate(
            [kh, np.ones((HPC, 1, S), np.float32)], axis=1)
        v_host = np.concatenate(
            [v[hs], np.ones((HPC, S, 1), np.float32)], axis=-1)
        in_maps.append({
            "qT": np.ascontiguousarray(qT_host),
            "kT": np.ascontiguousarray(kT_host),
            "v": _to_bf16(v_host).view(ml_dtypes.bfloat16),
        })

    res = run_bass_kernel_spmd(nc, in_maps, list(range(N_CORES)))

    outp = np.empty((B, H, S, D), dtype=np.float32)
    for c in range(N_CORES):
        for hh in range(HPC):
            outp[0, c * HPC + hh] = res.results[c]["out"][hh]
    return outp
